# revision 2
# baseline (speedup 1.0000x reference)
"""Causal self-attention Trainium2 Bass kernel, v2.

Problem: B=4, T=2048, C=2048, H=16 heads, D=128 head dim, fp32 I/O.
  qkv = x @ w_qkv ; causal softmax(q k^T / sqrt(D)) v ; out = av @ w_proj

Sharding (8 NeuronCores): DP=4 over batch x TP=2 over head groups
(Megatron-style). Core c: batch c//2, heads (c%2)*8..+8. Each core emits
a partial [T, C] fp32 output; host sums TP pairs.

v2 changes vs baseline:
  - fp16 activations/weights everywhere (tolerance 2e-2; fp32r and fp16
    matmuls both run 1 col/cycle, but fp16 halves SBUF/DMA and enables
    DVE 2x modes).
  - Fully SBUF-resident: no DRAM scratch round trips between phases.
  - Softmax denominators moved off the PE: DVE accumulates exp tiles
    (fp16 2x), gpsimd partition_all_reduce produces broadcast sums.
    No ones-matmul, no partition_broadcast.
  - exp batched over [128,1024] PSUM score-pair tiles (2 banks) to
    amortize ACT fixed overhead; diagonal pairs packed tight.
  - Stationary-reuse loop orders in QK/V/OUT phases (ldweights per 4/2/4
    matmuls instead of per matmul).

Phases: QK proj -> V proj -> attention (8 heads) -> out proj.
"""

import math
import os
import sys

import numpy as np

for _p in ("/opt/trn_rl_repo",):
    if _p not in sys.path:
        sys.path.insert(0, _p)

import concourse.bass as bass
import concourse.mybir as mybir
from concourse import bacc
from concourse import bass_isa
from concourse.tile import TileContext

B, T, C, H, D = 4, 2048, 2048, 16, 128
P = 128
NCORES = 8
HL = 8           # heads per core
FL = HL * D      # local feature dim = 1024
NCC = C // P     # 16 contraction chunks
NTC = T // P     # 16 t chunks
NSB = T // 512   # 4 t superblocks
EXP_SCALE = 1.0 / math.sqrt(D)

f32 = mybir.dt.float32
f16 = mybir.dt.float16


def _pair_layout(si, pr):
    """Packing of score pair pr (j-chunks 2pr, 2pr+1) of superblock si into a
    [128, 1024] PSUM tile. Returns [(jj, d_off, col_off, width), ...]."""
    infos = []
    js = (2 * pr, 2 * pr + 1)
    d0 = max(0, js[0] * P - si * 512)
    w0 = 512 - d0
    d1 = max(0, js[1] * P - si * 512)
    w1 = 512 - d1
    o1 = w0 if (w0 + w1) <= 512 else 512
    infos.append((js[0], d0, 0, w0))
    infos.append((js[1], d1, o1, w1))
    return infos


def build_nc():
    nc = bacc.Bacc()
    xt_d = nc.declare_dram_parameter("xt", [C, T], f16, isOutput=False)
    wqk_d = nc.declare_dram_parameter("wqk", [16, P, C], f16, isOutput=False)
    wv_d = nc.declare_dram_parameter("wv", [NCC, P, FL], f16, isOutput=False)
    wp_d = nc.declare_dram_parameter("wp", [HL, P, C], f16, isOutput=False)
    mask_d = nc.declare_dram_parameter("mask", [P, P], f16, isOutput=False)
    out_d = nc.declare_dram_parameter("out", [T, C], f32, isOutput=True)

    ACT = mybir.ActivationFunctionType

    with TileContext(nc) as tc:
        with tc.tile_pool(name="const", bufs=1) as cpool, \
             tc.tile_pool(name="qkp", bufs=1) as qk_pool, \
             tc.tile_pool(name="vp", bufs=1) as v_pool:
            mask_sb = cpool.tile([P, P], f16)
            nc.sync.dma_start(mask_sb[:], mask_d[:])
            qk = [qk_pool.tile([P, T], f16, tag=f"qk{j}", name=f"qk{j}")
                  for j in range(16)]
            vsb = [v_pool.tile([P, FL], f16, tag=f"v{tb}", name=f"v{tb}")
                   for tb in range(NTC)]

            cpi = 0  # copy-engine round robin

            with tc.tile_pool(name="xtp", bufs=1) as xt_pool, \
                 tc.tile_pool(name="wvp", bufs=1) as wv_pool:
                xts = [xt_pool.tile([P, T], f16, tag=f"xt{cc}", name=f"xt{cc}")
                       for cc in range(NCC)]
                wvs = [wv_pool.tile([P, FL], f16, tag=f"wv{cc}", name=f"wv{cc}")
                       for cc in range(NCC)]
                # Input DMAs: xt is on the critical path -> split across two
                # queues; wv needed only at V phase -> two more queues.
                for cc in range(8):
                    nc.sync.dma_start(xts[cc][:], xt_d[cc * P:(cc + 1) * P, :])
                for cc in range(8, NCC):
                    nc.gpsimd.dma_start(xts[cc][:], xt_d[cc * P:(cc + 1) * P, :])
                for cc in range(NCC):
                    nc.scalar.dma_start(wvs[cc][:], wv_d[cc])

                # ---------------- QK projection ----------------
                # out[j-block, t] = sum_cc wqk[jc-block].T @ xT ; [d, t] layout
                with tc.tile_pool(name="wqkp", bufs=3) as wqk_pool, \
                     tc.tile_pool(name="pp", bufs=8, space="PSUM") as pp:
                    jorder = [x for hh in range(HL) for x in (hh, HL + hh)]
                    for j in jorder:
                        wt = wqk_pool.tile([P, C], f16, tag="wqk")
                        nc.sync.dma_start(wt[:], wqk_d[j])
                        pss = [pp.tile([P, 512], f32, tag="pp",
                                       name=f"pp{j}_{sb}") for sb in range(NSB)]
                        for cc in range(NCC):
                            for sb in range(NSB):
                                nc.tensor.matmul(
                                    pss[sb][:], wt[:, cc * P:(cc + 1) * P],
                                    xts[cc][:, sb * 512:(sb + 1) * 512],
                                    start=(cc == 0), stop=(cc == NCC - 1))
                        for sb in range(NSB):
                            dst = qk[j][:, sb * 512:(sb + 1) * 512]
                            if cpi % 2 == 0:
                                nc.vector.tensor_copy(out=dst, in_=pss[sb][:])
                            else:
                                nc.scalar.copy(out=dst, in_=pss[sb][:])
                            cpi += 1

                    # ---------------- V projection ----------------
                    # out[t-chunk, vcol] natural layout
                    for tb in range(NTC):
                        pss = [pp.tile([P, 512], f32, tag="pp",
                                       name=f"pv{tb}_{vb}") for vb in range(2)]
                        for cc in range(NCC):
                            for vb in range(2):
                                nc.tensor.matmul(
                                    pss[vb][:], xts[cc][:, tb * P:(tb + 1) * P],
                                    wvs[cc][:, vb * 512:(vb + 1) * 512],
                                    start=(cc == 0), stop=(cc == NCC - 1))
                        for vb in range(2):
                            dst = vsb[tb][:, vb * 512:(vb + 1) * 512]
                            if cpi % 2 == 0:
                                nc.vector.tensor_copy(out=dst, in_=pss[vb][:])
                            else:
                                nc.scalar.copy(out=dst, in_=pss[vb][:])
                            cpi += 1
            # xts, wvs freed

            with tc.tile_pool(name="wpp", bufs=1) as wp_pool, \
                 tc.tile_pool(name="avtp", bufs=1) as avt_pool:
                wps = [wp_pool.tile([P, C], f16, tag=f"wp{f}", name=f"wp{f}")
                       for f in range(HL)]
                for f in range(HL):
                    nc.gpsimd.dma_start(wps[f][:], wp_d[f])
                avts = [avt_pool.tile([P, T], f16, tag=f"avt{h}",
                                      name=f"avt{h}") for h in range(HL)]

                # ---------------- attention ----------------
                with tc.tile_pool(name="scp", bufs=2, space="PSUM") as scp, \
                     tc.tile_pool(name="avp", bufs=2, space="PSUM") as avp, \
                     tc.tile_pool(name="etp", bufs=4) as etp, \
                     tc.tile_pool(name="Sp", bufs=2) as Sp, \
                     tc.tile_pool(name="stp", bufs=4) as stp:
                    for h in range(HL):
                        qt, kt = qk[h], qk[HL + h]
                        for si in range(NSB):
                            njc = 4 * si + 4
                            av_ps = avp.tile([P, 512], f32, tag="av")
                            S = Sp.tile([P, 512], f16, tag="S")
                            pend = None
                            for pr in range(njc // 2):
                                infos = _pair_layout(si, pr)
                                sc = scp.tile([P, 1024], f32, tag="sc")
                                for (jj, dd, oo, ww) in infos:
                                    nc.tensor.matmul(
                                        sc[:, oo:oo + ww],
                                        kt[:, jj * P:(jj + 1) * P],
                                        qt[:, si * 512 + dd:(si + 1) * 512],
                                        start=True, stop=True)
                                et = etp.tile([P, 1024], f16, tag="et")
                                end = infos[-1][2] + infos[-1][3]
                                nc.scalar.activation(
                                    et[:, :end], sc[:, :end], ACT.Exp,
                                    scale=EXP_SCALE)
                                for (jj, dd, oo, ww) in infos:
                                    if jj >= 4 * si:  # diagonal 128-block
                                        nc.vector.tensor_mul(
                                            out=et[:, oo:oo + P],
                                            in0=et[:, oo:oo + P],
                                            in1=mask_sb[:])
                                for (jj, dd, oo, ww) in infos:
                                    if jj == 0:
                                        nc.vector.tensor_copy(
                                            out=S[:], in_=et[:, 0:512])
                                    else:
                                        nc.vector.tensor_add(
                                            out=S[:, dd:], in0=S[:, dd:],
                                            in1=et[:, oo:oo + ww])
                                if pend is not None:
                                    pet, pinfos = pend
                                    for (jj, dd, oo, ww) in pinfos:
                                        nc.tensor.matmul(
                                            av_ps[:, dd:],
                                            vsb[jj][:, h * P:(h + 1) * P],
                                            pet[:, oo:oo + ww],
                                            start=(jj == 0), stop=False)
                                pend = (et, infos)
                            pet, pinfos = pend
                            for (jj, dd, oo, ww) in pinfos:
                                nc.tensor.matmul(
                                    av_ps[:, dd:],
                                    vsb[jj][:, h * P:(h + 1) * P],
                                    pet[:, oo:oo + ww],
                                    start=(jj == 0), stop=(jj == njc - 1))
                            Sb = stp.tile([P, 512], f32, tag="Sb")
                            nc.gpsimd.partition_all_reduce(
                                Sb[:], S[:], channels=P,
                                reduce_op=bass_isa.ReduceOp.add)
                            rec = stp.tile([P, 512], f32, tag="rec")
                            nc.vector.reciprocal_approx_fast(
                                out=rec[:], in_=Sb[:])
                            nc.vector.tensor_mul(
                                out=avts[h][:, si * 512:(si + 1) * 512],
                                in0=av_ps[:], in1=rec[:])

                # ---------------- output projection ----------------
                with tc.tile_pool(name="pop", bufs=8, space="PSUM") as pop, \
                     tc.tile_pool(name="obp", bufs=6) as obp:
                    for tch in range(NTC):
                        pss = [pop.tile([P, 512], f32, tag="po",
                                        name=f"po{tch}_{cb}")
                               for cb in range(C // 512)]
                        for f in range(HL):
                            for cb in range(C // 512):
                                nc.tensor.matmul(
                                    pss[cb][:],
                                    avts[f][:, tch * P:(tch + 1) * P],
                                    wps[f][:, cb * 512:(cb + 1) * 512],
                                    start=(f == 0), stop=(f == HL - 1))
                        for cb in range(C // 512):
                            ob = obp.tile([P, 512], f32, tag="ob")
                            if cpi % 2 == 0:
                                nc.vector.tensor_copy(out=ob[:], in_=pss[cb][:])
                            else:
                                nc.scalar.copy(out=ob[:], in_=pss[cb][:])
                            cpi += 1
                            nc.sync.dma_start(
                                out_d[tch * P:(tch + 1) * P,
                                      cb * 512:(cb + 1) * 512], ob[:])
    nc.compile()
    return nc


def _make_mask():
    pp_ = np.arange(P)[:, None]
    ff = np.arange(P)[None, :]
    return np.where(ff >= pp_, 1.0, 0.0).astype(np.float16)


def _prep_inputs(x, w_qkv, w_proj):
    mask = _make_mask()
    per_g = {}
    for g in range(2):
        q = w_qkv[:, g * FL:(g + 1) * FL]
        k = w_qkv[:, C + g * FL:C + (g + 1) * FL]
        v = w_qkv[:, 2 * C + g * FL:2 * C + (g + 1) * FL]
        wqk_cat = np.concatenate([q, k], axis=1)  # [C, 2048]
        wqk_p = np.ascontiguousarray(
            wqk_cat.reshape(NCC, P, 16, P).transpose(2, 1, 0, 3)
            .reshape(16, P, C)).astype(np.float16)
        wv_p = np.ascontiguousarray(v.reshape(NCC, P, FL)).astype(np.float16)
        wp_p = np.ascontiguousarray(
            w_proj[g * FL:(g + 1) * FL, :].reshape(HL, P, C)).astype(np.float16)
        per_g[g] = (wqk_p, wv_p, wp_p)
    in_maps = []
    for core in range(NCORES):
        b, g = core // 2, core % 2
        wqk_p, wv_p, wp_p = per_g[g]
        in_maps.append({
            "xt": np.ascontiguousarray(x[b].T).astype(np.float16),
            "wqk": wqk_p,
            "wv": wv_p,
            "wp": wp_p,
            "mask": mask,
        })
    return in_maps


_nc_cache = None
last_results = None  # BassKernelResults of the most recent run (for test.py)


def kernel(x, w_qkv, w_proj):
    global _nc_cache, last_results
    from concourse.bass_utils import run_bass_kernel_spmd

    x = np.asarray(x, dtype=np.float32)
    w_qkv = np.asarray(w_qkv, dtype=np.float32)
    w_proj = np.asarray(w_proj, dtype=np.float32)

    if _nc_cache is None:
        _nc_cache = build_nc()
    nc = _nc_cache

    in_maps = _prep_inputs(x, w_qkv, w_proj)
    trace = bool(int(os.environ.get("KERNEL_TRACE", "0")))
    res = run_bass_kernel_spmd(nc, in_maps, list(range(NCORES)), trace=trace)
    last_results = res

    out = np.empty((B, T, C), dtype=np.float32)
    for b in range(B):
        out[b] = res.results[2 * b]["out"] + res.results[2 * b + 1]["out"]
    return out
